# revision 13
# baseline (speedup 1.0000x reference)
"""Trainium2 kernel for nn_MultiHeadClassifier.

Math: out[i] = W[task_labels[i]] @ x[i] + b[task_labels[i]]
  x [262144, 1024] f32, task_labels [262144] int, W [8, 32, 1024], b [8, 32]

Strategy (8 NeuronCores, data-parallel over batch), v5:
  - The problem is HBM-bound: the only large tensor is x. v1 streamed x as
    f32 (128 MiB/core) and computed all 8 heads on the PE, selecting via a
    one-hot mask (8x the needed matmul work). v2+ halves the traffic and
    cuts PE work 8x:
      * x is cast to bf16 on the host (tolerance is 2e-2; bf16 adds ~2.6e-3).
      * Rows are routed on the host: each core's 32768 rows are placed
        into 8 static 4096-row blocks by task id. The device schedule is
        fully static: rows in block t use W[t]. No masks, no padding.
      * Block overflow (a task with >4096 rows on one core; ~24 rows
        expected per block) is computed on the host in numpy and patched
        into the output. Underfull blocks hold zero rows (harmless).
  - Device inner loop: per 512-row chunk, 8 accumulating matmuls with the
    block's W as the stationary operand ([128k, 32], N=512 moving rows
    from the [ki, rows]-transposed x), psum [32, 512] -> DVE copy/cast to
    bf16 -> DMA out as [32, rows].
  - v5 pipeline shape: the kernel end is PE-drain-limited (compute of the
    final superblock can only start once its last byte lands), so x
    streams as 2 MB superblocks with 512-row pieces at both ends: the
    prologue pieces ride the ACT ring (both HWDGE rings generate
    descriptors in parallel during engine init) and the tail pieces keep
    the final PE trail to ~1 chunk.
  - Host: inverse permutation, bias add, f32 cast.
"""

import sys

sys.path.insert(0, "/opt/trn_rl_repo")

import numpy as np
import ml_dtypes

import concourse.bass as bass
import concourse.tile as tile
from concourse import bacc, mybir
from concourse import bass_utils

B, D, C, T = 262144, 1024, 32, 8
NCORES = 8
N = B // NCORES  # 32768 rows per core
P = 128
KO = D // P  # 8 contraction tiles
BLK = N // T  # 4096 rows per task block (static capacity)
SB = 1024  # rows per mid superblock (one x DMA = 2 MB)
CHUNK = 512  # rows per psum accumulation group
NPRO = 2  # 512-row prologue pieces
NTAIL = 2  # 512-row tail pieces
ROW0 = NPRO * CHUNK  # first mid row
ROW1 = N - NTAIL * CHUNK  # first tail row
NMID = (ROW1 - ROW0) // SB  # 30 mid superblocks
NCH = SB // CHUNK  # chunks per mid superblock

# set by test harness to collect a profile; harness-invoked kernel() keeps it off
TRACE = False
LAST_RESULTS = None


def _build():
    f32 = mybir.dt.float32
    bf16 = mybir.dt.bfloat16

    nc = bacc.Bacc("TRN2", debug=False, num_devices=NCORES)
    # xt[sb, ki, ko, r]: rows already routed into task blocks; 16 KB
    # contiguous per partition per superblock -> near-peak DMA efficiency.
    xt_d = nc.dram_tensor("xt", [NMID, P, KO, SB], bf16, kind="ExternalInput")
    # piece-major ends of the stream (prologue + tail)
    xp_d = nc.dram_tensor(
        "xp", [NPRO + NTAIL, P, KO, CHUNK], bf16, kind="ExternalInput"
    )
    # wall[ki, t, ko, c] = W[t, c, ko*128+ki] (lhsT layout, all 8 heads)
    wall_d = nc.dram_tensor("wall", [P, T, KO, C], bf16, kind="ExternalInput")
    out_d = nc.dram_tensor("out", [C, N], bf16, kind="ExternalOutput")

    with tile.TileContext(nc) as tc:
        with (
            tc.tile_pool(name="sbuf", bufs=1) as sbuf,
            tc.tile_pool(name="xpool", bufs=4) as xpool,
            tc.tile_pool(name="psum", bufs=8, space="PSUM") as psum,
        ):
            # consts + prologue pieces on the ACT ring; mids start on the
            # SP ring concurrently (parallel descriptor generation)
            wall = sbuf.tile([P, T, KO, C], bf16)
            nc.scalar.dma_start(wall[:], wall_d[:])
            pros = []
            for i in range(NPRO):
                xpc = xpool.tile([P, KO, CHUNK], bf16, tag="xp")
                nc.scalar.dma_start(xpc[:], xp_d[i])
                pros.append(xpc)
            xts_list = []
            for i in range(min(2, NMID)):
                xts = xpool.tile([P, KO, SB], bf16, tag="xts")
                nc.sync.dma_start(xts[:], xt_d[i])
                xts_list.append(xts)

            # Engine warmups: one instruction per engine that observes the
            # const DMA lane, so steady-state instructions carry at most
            # one semaphore wait each.
            scratch = psum.tile([C, CHUNK], f32, tag="y")
            nc.tensor.matmul(
                scratch[:2, :2], wall[:2, 0, 0, :2], wall[:2, 0, 0, :2],
                start=True, stop=True,
            )
            dve_scr = sbuf.tile([P, C], bf16, tag="dve_scr")
            nc.vector.tensor_copy(dve_scr[:], wall[:, 0, 0, :])

            def chunk_group(xap, t, out_slice):
                y = psum.tile([C, CHUNK], f32, tag="y")
                for ko in range(KO):
                    nc.tensor.matmul(
                        y[:],
                        wall[:, t, ko, :],
                        xap(ko),
                        start=(ko == 0),
                        stop=(ko == KO - 1),
                    )
                nc.vector.tensor_copy(out_slice, y[:])

            # prologue compute (rows [0, ROW0) are all task 0)
            for i in range(NPRO):
                out_p = xpool.tile([C, CHUNK], bf16, tag="out_p")
                chunk_group(lambda ko: pros[i][:, ko, :], 0, out_p[:])
                nc.scalar.dma_start(
                    out_d[:, i * CHUNK : (i + 1) * CHUNK], out_p[:]
                )

            for sb in range(NMID):
                r0 = ROW0 + sb * SB
                if sb < 2:
                    xts = xts_list[sb]
                else:
                    xts = xpool.tile([P, KO, SB], bf16, tag="xts")
                    nc.sync.dma_start(xts[:], xt_d[sb])
                out_sb = xpool.tile([C, SB], bf16, tag="out_sb")
                for st in range(NCH):
                    t = (r0 + st * CHUNK) // BLK  # static task id
                    chunk_group(
                        lambda ko: xts[:, ko, st * CHUNK : (st + 1) * CHUNK],
                        t,
                        out_sb[:, st * CHUNK : (st + 1) * CHUNK],
                    )
                # out on the ACT HWDGE ring so it never delays xts loads
                nc.scalar.dma_start(out_d[:, r0 : r0 + SB], out_sb[:])

            # tail pieces keep the final PE trail to one chunk
            for i in range(NTAIL):
                r0 = ROW1 + i * CHUNK
                xpc = xpool.tile([P, KO, CHUNK], bf16, tag="xp")
                nc.sync.dma_start(xpc[:], xp_d[NPRO + i])
                out_p = xpool.tile([C, CHUNK], bf16, tag="out_p")
                chunk_group(lambda ko: xpc[:, ko, :], (r0 // BLK), out_p[:])
                nc.scalar.dma_start(out_d[:, r0 : r0 + CHUNK], out_p[:])
    nc.compile()
    return nc


_NC = None


def _get_nc():
    global _NC
    if _NC is None:
        _NC = _build()
    return _NC


def kernel(x, task_labels, W, b):
    global LAST_RESULTS
    x = np.asarray(x)
    if x.dtype != np.float32:
        x = x.astype(np.float32)
    labels = np.asarray(task_labels).astype(np.int64)
    W32 = np.asarray(W)
    if W32.dtype != np.float32:
        W32 = W32.astype(np.float32)
    b32 = np.asarray(b)
    if b32.dtype != np.float32:
        b32 = b32.astype(np.float32)

    wall = np.ascontiguousarray(
        W32.reshape(T, C, KO, P).transpose(3, 0, 2, 1)
    ).astype(ml_dtypes.bfloat16)

    in_maps = []
    placements = []
    for c in range(NCORES):
        lab = labels[c * N : (c + 1) * N]
        xs16 = x[c * N : (c + 1) * N].astype(ml_dtypes.bfloat16)
        slot_to_row = np.full(N, -1, np.int64)
        overflow = []
        for t in range(T):
            idx = np.nonzero(lab == t)[0]
            n_place = min(len(idx), BLK)
            slot_to_row[t * BLK : t * BLK + n_place] = idx[:n_place]
            if len(idx) > BLK:
                overflow.append(idx[BLK:])
        placed = slot_to_row >= 0
        xb = np.zeros((N, D), ml_dtypes.bfloat16)
        xb[placed] = xs16[slot_to_row[placed]]
        # xt[sb, ki, ko, r] = xb[sb*SB + r, ko*P + ki]
        xt = np.ascontiguousarray(
            xb[ROW0:ROW1].reshape(NMID, SB, KO, P).transpose(0, 3, 2, 1)
        )
        xpieces = np.concatenate([xb[:ROW0], xb[ROW1:]])
        xp = np.ascontiguousarray(
            xpieces.reshape(NPRO + NTAIL, CHUNK, KO, P).transpose(0, 3, 2, 1)
        )
        in_maps.append({"xt": xt, "xp": xp, "wall": wall})
        placements.append(
            (
                slot_to_row,
                placed,
                np.concatenate(overflow) if overflow else np.empty(0, np.int64),
            )
        )

    nc = _get_nc()
    res = bass_utils.run_bass_kernel_spmd(
        nc, in_maps, core_ids=list(range(NCORES)), trace=TRACE
    )
    LAST_RESULTS = res

    out = np.empty((B, C), np.float32)
    for c in range(NCORES):
        dev = np.asarray(res.results[c]["out"]).astype(np.float32).T  # [N, C]
        slot_to_row, placed, overflow = placements[c]
        rows = slot_to_row[placed]
        out[c * N + rows] = dev[placed]
        if len(overflow):
            lab = labels[c * N : (c + 1) * N]
            xs = x[c * N : (c + 1) * N]
            for t in np.unique(lab[overflow]):
                rr = overflow[lab[overflow] == t]
                out[c * N + rr] = xs[rr] @ W32[t].T
    out += b32[labels]
    return out
